# revision 1
# baseline (speedup 1.0000x reference)
"""Trainium2 Bass kernel for order-2 sign-residual binarization with
alternating refinement (vq_codebook problem).

Full inputs -> shard rows across 8 NeuronCores -> SPMD Bass kernel -> gather.

Algorithm (per-core shard of rows):  the reference's scan iteration only
needs six per-row statistics of the static masked tensor x:
    N_j = #{x > phi_j},  S_j = sum relu(x - phi_j),  j in {1,2,3}
at thresholds (phi1, phi2, phi3) = (rmean - a0, rmean, rmean + a0): the
4-candidate argmin classification is a 3-threshold partition of x
(canonical ordering a0 > a1 > 0 holds for this data; validated
numerically).  Each stat is ONE fused pass (tensor_scalar with accum_out,
2x_2p f32 mode on DVE / activation accum on ScalarE) over the single
resident tensor  xbig = where(mask, x, -1000).  Per-row scalar
recurrences (mu, rmean, a0, a1) run as [128,1] ops on GPSIMD so the DVE/
ACT stay on the big passes.  The final iteration assembles
    out = v0*maskf + 2*a1*(ind1 + ind3) + 2*(a0-a1)*ind2
from scaled-indicator passes and DMAs it out.

Engine budget per tile-iteration: DVE 4 fused passes (N1,N2,N3 and
S2 via sum-max), ACT 2 relu-accum passes (S1,S3), GPSIMD the scalar
chain.  (TensorScalarPtr with accum_out is not a valid Pool opcode, so
GPSIMD cannot take a fused pass.)
"""

import numpy as np

import concourse.bacc as bacc
import concourse.tile as tile
from concourse import mybir
from concourse.bass_utils import run_bass_kernel_spmd

A = mybir.AluOpType
F32 = mybir.dt.float32
AX = mybir.AxisListType.X
RELU = mybir.ActivationFunctionType.Relu

N_CORES = 8
R_FULL, C_FULL = 4096, 11008
NEG_BIG = -1000.0
ACT_OUT_BF16 = False  # write ACT relu outputs as bf16 (accum stays f32)
BF16 = mybir.dt.bfloat16


def build_module(rows, C, nch, num_iters, xb_bufs=2, scr_bufs=6):
    """Build the Bass module for one core's shard of `rows` rows."""
    assert rows % 128 == 0 and C % nch == 0
    P = 128
    CH = C // nch
    ntiles = rows // P

    nc = bacc.Bacc("TRN2", target_bir_lowering=False, debug=False,
                   enable_asserts=False)
    xbig_d = nc.dram_tensor("xbig", [rows, C], F32, kind="ExternalInput").ap()
    stats_d = nc.dram_tensor("rowstats", [rows, 2], F32,
                             kind="ExternalInput").ap()
    out_d = nc.dram_tensor("out", [rows, C], F32, kind="ExternalOutput").ap()

    with tile.TileContext(nc) as tc:
        with (
            tc.tile_pool(name="xp", bufs=xb_bufs) as xpool,
            tc.tile_pool(name="scr", bufs=scr_bufs) as scrpool,
            tc.tile_pool(name="acc", bufs=12) as accpool,
            tc.tile_pool(name="sc", bufs=4) as spool,
            tc.tile_pool(name="one", bufs=1) as singles,
        ):
            c_m500 = singles.tile([P, 1], F32, tag="c_m500", name="c_m500")
            nc.vector.memset(c_m500[:], NEG_BIG / 2.0)
            # Dead-value dump buffers for fused accum passes.  Monolithic
            # passes write here; same-engine FIFO order makes reuse safe,
            # and bf16 halves the footprint (accumulate happens pre-cast).
            act_dump = singles.tile([P, C], BF16, tag="act_dump",
                                    name="act_dump")
            dve_dump = singles.tile([P, C], BF16, tag="dve_dump",
                                    name="dve_dump")

            def sc(tag):
                return spool.tile([P, 1], F32, tag=tag, name=tag)

            def tt(in0, in1, op, tag, engine=None):
                o = sc(tag)
                (engine or nc.gpsimd).tensor_tensor(o[:], in0[:], in1[:], op)
                return o

            def ts(in0, s1, op0, tag, s2=None, op1=None, engine=None):
                """o = (in0 op0 s1) [op1 s2]; s1/s2 float or [128,1] tile."""
                o = sc(tag)
                kw = {}
                if op1 is not None:
                    kw["op1"] = op1
                if s2 is not None and not isinstance(s2, float):
                    s2 = s2[:]
                if not isinstance(s1, float):
                    s1 = s1[:]
                (engine or nc.gpsimd).tensor_scalar(o[:], in0[:], s1, s2,
                                                    op0=op0, **kw)
                return o

            def emit_pass(xb, phi, op0, tag, engine):
                """accum-reduce of (xbig op0 phi) over the row -> [128,1]."""
                o = sc(tag)
                engine.tensor_scalar(dve_dump[:], xb[:], phi[:], None,
                                     op0=op0, op1=A.add, accum_out=o[:])
                return o

            def emit_count(xb, phi, tag):
                """N = #{xbig > phi} per row (mask folded into xbig). DVE."""
                return emit_pass(xb, phi, A.is_gt, tag, nc.vector)

            def emit_relusum_act(xb, nphi, tag):
                """S = sum relu(xbig - phi), one monolithic ACT op."""
                o = sc(tag)
                nc.scalar.activation(act_dump[:], xb[:], RELU, bias=nphi[:],
                                     scale=1.0, accum_out=o[:])
                return o

            def fin(acc_or_s, tag):
                """Finalize a deferred accumulator into a [P,1] scalar."""
                if acc_or_s.shape[1] == 1:
                    return acc_or_s
                o = sc(tag)
                nc.vector.tensor_reduce(o[:], acc_or_s[:], axis=AX, op=A.add)
                return o

            def emit_relusum_dve(xb, phi, tag):
                """S via DVE: sum max(xbig,phi), minus C*phi (phi ~ 0 so the
                accumulator stays small and precise)."""
                m = emit_pass(xb, phi, A.max, tag + "_m", nc.vector)
                cphi = ts(phi, float(C), A.mult, tag + "_cp")
                return tt(m, cphi, A.subtract, tag)

            def neg(x, tag):
                return ts(x, -1.0, A.mult, tag)

            dma_engines = [nc.sync, nc.scalar, nc.sync, nc.scalar]

            def phase_load(t):
                r0 = t * P
                xb = xpool.tile([P, C], F32, tag="xb", name="xb")
                for c in range(nch):
                    dma_engines[c % len(dma_engines)].dma_start(
                        out=xb[:, c * CH:(c + 1) * CH],
                        in_=xbig_d[r0:r0 + P, c * CH:(c + 1) * CH])
                st = spool.tile([P, 2], F32, tag="st", name="st")
                nc.sync.dma_start(out=st[:], in_=stats_d[r0:r0 + P, :])
                cnt = sc("cnt")
                nc.gpsimd.tensor_copy(cnt[:], st[:, 0:1])
                Sx = sc("Sx")
                nc.gpsimd.tensor_copy(Sx[:], st[:, 1:2])
                cntm1 = ts(cnt, 1.0, A.max, "cntm1")
                rc = sc("rc")
                nc.vector.reciprocal(rc[:], cntm1[:])
                return dict(r0=r0, xb=xb, cnt=cnt, Sx=Sx, rc=rc)

            def phase_order1(s):
                xb, cnt, Sx, rc = s["xb"], s["cnt"], s["Sx"], s["rc"]
                mu0 = tt(Sx, rc, A.mult, "mu0")
                nmu0 = neg(mu0, "nmu0")
                s["N2"] = emit_count(xb, mu0, "N2o")
                s["S2"] = emit_relusum_act(xb, nmu0, "S2o")
                s["mu0"] = mu0

            def phase_order2(s):
                xb, cnt, Sx, rc = s["xb"], s["cnt"], s["Sx"], s["rc"]
                mu0, N2 = s["mu0"], s["N2"]
                S2 = s["S2"] = fin(s["S2"], "S2f")
                # alpha0 = (2*S2 - Sx + mu0*cnt) / cnt
                h1 = tt(mu0, cnt, A.mult, "h1")
                h3 = ts(S2, 2.0, A.mult, "h3", s2=Sx, op1=A.subtract)
                h4 = tt(h3, h1, A.add, "h4")
                alpha0 = tt(h4, rc, A.mult, "a0")
                # Sb0 = 2*N2 - cnt ; mu1 = (Sx - mu0*cnt - alpha0*Sb0)/cnt
                Sb0o = ts(N2, 2.0, A.mult, "Sb0o", s2=cnt, op1=A.subtract)
                h6 = tt(Sx, h1, A.subtract, "h6")
                h7 = tt(alpha0, Sb0o, A.mult, "h7")
                h8 = tt(h6, h7, A.subtract, "h8")
                mu1 = tt(h8, rc, A.mult, "mu1")
                rmean = tt(mu0, mu1, A.add, "rmean")
                phi1 = tt(rmean, alpha0, A.subtract, "phi1")
                phi3 = tt(rmean, alpha0, A.add, "phi3")
                nphi1 = neg(phi1, "nphi1")
                nphi3 = neg(phi3, "nphi3")
                s["N1"] = emit_count(xb, phi1, "N1o")
                s["S1"] = emit_relusum_act(xb, nphi1, "S1o")
                s["N3"] = emit_count(xb, phi3, "N3o")
                s["S3"] = emit_relusum_act(xb, nphi3, "S3o")
                s.update(alpha0=alpha0, rmean=rmean, phi1=phi1, phi3=phi3,
                         phi2=mu0)

            def phase_order3(s):
                cnt, Sx, rc = s["cnt"], s["Sx"], s["rc"]
                N1, N2, N3 = s["N1"], s["N2"], s["N3"]
                S1 = s["S1"] = fin(s["S1"], "S1f")
                S2 = s["S2"]
                S3 = s["S3"] = fin(s["S3"], "S3f")
                phi1, phi2, phi3 = s["phi1"], s["phi2"], s["phi3"]
                alpha0 = s["alpha0"]
                W1 = ts(N1, phi1, A.mult, "W1", s2=S1, op1=A.add)
                W2 = ts(N2, phi2, A.mult, "W2", s2=S2, op1=A.add)
                W3 = ts(N3, phi3, A.mult, "W3", s2=S3, op1=A.add)
                # up = S3 + phi3*(N2-N3) - (W2-W3)
                u1 = tt(N2, N3, A.subtract, "u1")
                u2 = ts(u1, phi3, A.mult, "u2", s2=S3, op1=A.add)
                u4 = tt(W2, W3, A.subtract, "u4")
                up = tt(u2, u4, A.subtract, "up")
                # lo = 2*W1 - W2 - Sx + phi1*(cnt - 2*N1 + N2)
                l1 = ts(N1, -2.0, A.mult, "l1", s2=cnt, op1=A.add)
                l3 = tt(l1, N2, A.add, "l3")
                l4 = ts(l3, phi1, A.mult, "l4")
                l5 = ts(W1, 2.0, A.mult, "l5", s2=W2, op1=A.subtract)
                l7 = tt(l5, Sx, A.subtract, "l7")
                lo = tt(l7, l4, A.add, "lo")
                a1n = tt(up, lo, A.add, "a1n")
                alpha1 = tt(a1n, rc, A.mult, "a1")
                s["a0"], s["a1"] = alpha0, alpha1

            def phase_scalars(s):
                cnt, Sx, rc = s["cnt"], s["Sx"], s["rc"]
                N1, N2, N3 = s["N1"], s["N2"], s["N3"]
                S1 = fin(s["S1"], "S1f")
                S2 = fin(s["S2"], "S2f")
                S3 = fin(s["S3"], "S3f")
                phi1, phi2, phi3 = s["phi1"], s["phi2"], s["phi3"]
                a0, a1, rmean = s["a0"], s["a1"], s["rmean"]
                Sb0 = ts(N2, 2.0, A.mult, "Sb0", s2=cnt, op1=A.subtract)
                su = tt(N1, N3, A.add, "su")
                sv = ts(su, N2, A.subtract, "sv", s2=2.0, op1=A.mult)
                Sb1 = tt(sv, cnt, A.subtract, "Sb1")
                p1 = tt(N3, N1, A.subtract, "p1")
                Pst = ts(p1, 2.0, A.mult, "Pst", s2=cnt, op1=A.add)
                W1 = ts(N1, phi1, A.mult, "W1", s2=S1, op1=A.add)
                W2 = ts(N2, phi2, A.mult, "W2", s2=S2, op1=A.add)
                W3 = ts(N3, phi3, A.mult, "W3", s2=S3, op1=A.add)
                T0 = ts(W2, 2.0, A.mult, "T0", s2=Sx, op1=A.subtract)
                t2 = tt(W1, W3, A.add, "t2")
                t3 = ts(t2, W2, A.subtract, "t3", s2=2.0, op1=A.mult)
                T1 = tt(t3, Sx, A.subtract, "T1")
                # mu = (Sx - a0*Sb0 - a1*Sb1 - rmean*cnt)/cnt
                m1 = tt(a0, Sb0, A.mult, "m1")
                m2 = tt(a1, Sb1, A.mult, "m2")
                m3 = tt(rmean, cnt, A.mult, "m3")
                m4 = tt(Sx, m1, A.subtract, "m4")
                m5 = tt(m4, m2, A.subtract, "m5")
                m6 = tt(m5, m3, A.subtract, "m6")
                mu = tt(m6, rc, A.mult, "mu")
                rmean = tt(rmean, mu, A.add, "rmean")
                # a0' = (T0 - rmean*Sb0 - a1*P)/cnt
                q1 = tt(rmean, Sb0, A.mult, "q1")
                q2 = tt(T0, q1, A.subtract, "q2")
                q3 = tt(a1, Pst, A.mult, "q3")
                q4 = tt(q2, q3, A.subtract, "q4")
                a0 = tt(q4, rc, A.mult, "a0")
                # a1' = (T1 - rmean*Sb1 - a0'*P)/cnt
                r1_ = tt(rmean, Sb1, A.mult, "r1_")
                r2_ = tt(T1, r1_, A.subtract, "r2_")
                r3_ = tt(a0, Pst, A.mult, "r3_")
                r4_ = tt(r2_, r3_, A.subtract, "r4_")
                a1 = tt(r4_, rc, A.mult, "a1")
                phi1 = tt(rmean, a0, A.subtract, "phi1")
                phi3 = tt(rmean, a0, A.add, "phi3")
                s.update(a0=a0, a1=a1, rmean=rmean,
                         phi1=phi1, phi2=rmean, phi3=phi3)

            def phase_passes(s):
                xb = s["xb"]
                phi1, phi2, phi3 = s["phi1"], s["phi2"], s["phi3"]
                nphi1 = neg(phi1, "nphi1")
                nphi2 = neg(phi2, "nphi2")
                nphi3 = neg(phi3, "nphi3")
                s["N1"] = emit_count(xb, phi1, "N1")
                s["N2"] = emit_count(xb, phi2, "N2")
                s["N3"] = emit_count(xb, phi3, "N3")
                s["S1"] = emit_relusum_act(xb, nphi1, "S1")
                s["S3"] = emit_relusum_act(xb, nphi3, "S3")
                s["S2"] = emit_relusum_act(xb, nphi2, "S2")

            def phase_output(s):
                xb, r0 = s["xb"], s["r0"]
                phi1, phi2, phi3 = s["phi1"], s["phi2"], s["phi3"]
                a0, a1, rmean = s["a0"], s["a1"], s["rmean"]
                # out = v0*maskf + dv1*(ind1+ind3) + dv2*ind2
                dv1 = ts(a1, 2.0, A.mult, "dv1")
                dv2 = ts(a0, a1, A.subtract, "dv2", s2=2.0, op1=A.mult)
                v1_ = tt(rmean, a0, A.subtract, "v1_")
                v0 = tt(v1_, a1, A.subtract, "v0")
                for c in range(nch):
                    sl = xb[:, c * CH:(c + 1) * CH]
                    tA = scrpool.tile([P, CH], F32, tag="scr", name="scr")
                    nc.vector.tensor_scalar(tA[:], sl, phi1[:], dv1[:],
                                            op0=A.is_gt, op1=A.mult)
                    tB = scrpool.tile([P, CH], F32, tag="scr", name="scr")
                    nc.vector.tensor_scalar(tB[:], sl, phi3[:], dv1[:],
                                            op0=A.is_gt, op1=A.mult)
                    s1_ = scrpool.tile([P, CH], F32, tag="scr", name="scr")
                    nc.vector.tensor_tensor(s1_[:], tA[:], tB[:], A.add)
                    tC = scrpool.tile([P, CH], F32, tag="scr", name="scr")
                    nc.vector.tensor_scalar(tC[:], sl, phi2[:], dv2[:],
                                            op0=A.is_gt, op1=A.mult)
                    tM = scrpool.tile([P, CH], F32, tag="scr", name="scr")
                    nc.gpsimd.tensor_scalar(tM[:], sl, c_m500[:], v0[:],
                                            op0=A.is_gt, op1=A.mult)
                    s2_ = scrpool.tile([P, CH], F32, tag="scr", name="scr")
                    nc.gpsimd.tensor_tensor(s2_[:], tC[:], tM[:], A.add)
                    o_ = scrpool.tile([P, CH], F32, tag="scr", name="scr")
                    eng = nc.vector if c % 2 == 0 else nc.gpsimd
                    eng.tensor_tensor(o_[:], s1_[:], s2_[:], A.add)
                    dma_engines[c % len(dma_engines)].dma_start(
                        out=out_d[r0:r0 + P, c * CH:(c + 1) * CH], in_=o_[:])

            # Drive resident tiles pairwise in lockstep so each tile's
            # scalar chain hides under the other's elementwise passes.
            for t0 in range(0, ntiles, xb_bufs):
                pair = list(range(t0, min(t0 + xb_bufs, ntiles)))
                ss = [phase_load(t) for t in pair]
                for s in ss:
                    phase_order1(s)
                for s in ss:
                    phase_order2(s)
                for s in ss:
                    phase_order3(s)
                for k in range(1, num_iters + 1):
                    for s in ss:
                        phase_scalars(s)
                        if k < num_iters:
                            phase_passes(s)
                        else:
                            phase_output(s)
    nc.compile()
    return nc


_CACHE = {}


def _get_module(rows, C, nch, num_iters):
    key = (rows, C, nch, num_iters)
    if key not in _CACHE:
        _CACHE[key] = build_module(rows, C, nch, num_iters)
    return _CACHE[key]


def kernel(x, mask, order, num_iters):
    assert int(order) == 2
    num_iters = int(num_iters)
    x = np.ascontiguousarray(x, dtype=np.float32)
    maskb = np.asarray(mask, dtype=bool)
    R, C = x.shape

    xbig = np.where(maskb, x, np.float32(NEG_BIG)).astype(np.float32)
    maskf64 = maskb.astype(np.float64)
    cnt = maskf64.sum(1).astype(np.float32)
    Sx = (x.astype(np.float64) * maskf64).sum(1).astype(np.float32)
    rowstats = np.stack([cnt, Sx], axis=1).astype(np.float32)

    rows = R // N_CORES
    nc = _get_module(rows, C, 4, num_iters)

    in_maps = []
    for i in range(N_CORES):
        sl = slice(i * rows, (i + 1) * rows)
        in_maps.append({
            "xbig": np.ascontiguousarray(xbig[sl]),
            "rowstats": np.ascontiguousarray(rowstats[sl]),
        })
    res = run_bass_kernel_spmd(nc, in_maps, core_ids=list(range(N_CORES)))
    globals()["LAST_RESULT"] = res
    out = np.concatenate([r["out"] for r in res.results], axis=0)
    return out.astype(np.float32)


if __name__ == "__main__":
    rng = np.random.default_rng(0)
    x = (rng.standard_normal((R_FULL, C_FULL)) * 0.02).astype(np.float32)
    mask = rng.integers(0, 2, (R_FULL, C_FULL)) > 0
    out = kernel(x, mask, 2, 15)
    print(out.shape, out.dtype)

